# revision 9
# baseline (speedup 1.0000x reference)
"""Trainium2 Bass kernel for CausalBiasingNetwork bias computation.

bias[b,s,t] = sum_r (hs[b,s]@wc_r + bc_r)*strength_r * (hs[b,t]@we_r)
             + hs[b,t] @ be.sum(0)

Folded into a rank-17 form: append rule r=16 with wc=0, bc=1, strength=1,
we=be.sum(0).  Then with
    scaledT[r,s] = (hs[b,s] @ wc'_r + bc'_r) * strength'_r      [17, S]
    uT[r,t]     = hs[b,t] @ we'_r                               [17, S]
    bias[b]     = scaledT.T @ uT                                [S, S]

The K=17 bias matmuls are packed 4-at-a-time into the PE array via
tile_position row-tiling (strips at partitions 0/32/64/96).  To feed the
strips, uT is produced already replicated at all four partition bases
(the u-weights are replicated host-side, so the replication is free in
the A-stage matmul), and scaledT chunk q of each 512-column group is
stored at base 32*q — which is exactly the strip that s-tile uses.

Sharding: 8 cores = 4 batches x 2 sequence halves.  Core (b, h) receives
hs[b]^T (h-major, as the PE contraction needs) rolled so its 2048 output
rows come first; it computes out[s, t_rolled] and the host un-rolls the
columns when assembling the full [4, 4096, 4096] output.
"""

import contextlib

import ml_dtypes
import numpy as np

import concourse.bacc as bacc
import concourse.mybir as mybir
import concourse.tile as tile
from concourse.bass_utils import run_bass_kernel_spmd

B, S, H, R = 4, 4096, 1024, 16
R1 = R + 1          # 17 rules after folding the be-bias term
SH = S // 2         # 2048 output rows per core
P = 128             # partitions
TG = 512            # t-group width (one psum bank of f32)
N_TG = S // TG      # 8 t-groups
N_STILE = SH // P   # 16 s-tiles per core
F32 = mybir.dt.float32
BF16 = mybir.dt.bfloat16


def _emit(tc, aps):
    nc = tc.nc
    hst, wu4, ws4, smul4, sadd4, out = (
        aps["hst"], aps["wu4"], aps["ws4"], aps["smul4"], aps["sadd4"],
        aps["out"],
    )

    with contextlib.ExitStack() as ctx:
        consts = ctx.enter_context(tc.tile_pool(name="consts", bufs=1))
        hst_pool = ctx.enter_context(tc.tile_pool(name="hst", bufs=24))
        big_pool = ctx.enter_context(tc.tile_pool(name="big", bufs=1))
        out_pool = ctx.enter_context(tc.tile_pool(name="out", bufs=8))
        mm_ps = ctx.enter_context(
            tc.tile_pool(name="mm_ps", bufs=2, space="PSUM"))
        b_ps = ctx.enter_context(
            tc.tile_pool(name="b_ps", bufs=6, space="PSUM"))

        # ---- constants ----
        wu4_sb = consts.tile([P, 8 * P], BF16)   # u-weights, 4x replicated
        ws4_sb = consts.tile([P, 8 * P], BF16)   # s-weights, 4x replicated
        for c in range(8):
            nc.sync.dma_start(wu4_sb[:, c * P:(c + 1) * P],
                              wu4[c * P:(c + 1) * P, :])
            nc.sync.dma_start(ws4_sb[:, c * P:(c + 1) * P],
                              ws4[c * P:(c + 1) * P, :])
        smul_sb = consts.tile([P, 1], F32)
        sadd_sb = consts.tile([P, 1], F32)
        nc.sync.dma_start(smul_sb[:], smul4)
        nc.sync.dma_start(sadd_sb[:], sadd4)

        ut_sb = big_pool.tile([P, S], BF16)      # uT at bases 0/32/64/96
        st_sb = big_pool.tile([P, SH], BF16)     # scaledT, chunk q at base 32q

        def stage_a(tg):
            """Load hsT tiles for t-group tg; compute uT (+scaledT)."""
            cols = slice(tg * TG, (tg + 1) * TG)
            hsTt = []
            for hc in range(8):
                h = hst_pool.tile([P, TG], BF16, tag="hst")
                nc.sync.dma_start(
                    h[:], hst[hc * P:(hc + 1) * P, cols])
                hsTt.append(h)
            u_ps = mm_ps.tile([P, TG], F32, tag="mm")
            for hc in range(8):
                nc.tensor.matmul(
                    u_ps[:], wu4_sb[:, hc * P:(hc + 1) * P], hsTt[hc][:],
                    start=(hc == 0), stop=(hc == 7),
                )
            nc.scalar.copy(ut_sb[:, cols], u_ps[:])
            if tg < N_TG // 2:
                s_ps = mm_ps.tile([P, TG], F32, tag="mm")
                for hc in range(8):
                    nc.tensor.matmul(
                        s_ps[:], ws4_sb[:, hc * P:(hc + 1) * P], hsTt[hc][:],
                        start=(hc == 0), stop=(hc == 7),
                    )
                for q in range(4):
                    b0 = 32 * q
                    st = tg * 4 + q
                    nc.vector.tensor_scalar(
                        st_sb[b0:b0 + R1, st * P:(st + 1) * P],
                        s_ps[b0:b0 + R1, q * P:(q + 1) * P],
                        smul_sb[b0:b0 + R1, :], sadd_sb[b0:b0 + R1, :],
                        mybir.AluOpType.mult, mybir.AluOpType.add,
                    )

        def stage_b(tg):
            """All 16 bias s-tiles for t-columns of group tg + store."""
            cols = slice(tg * TG, (tg + 1) * TG)
            for g in range(4):
                bps = []
                for q in range(4):
                    st = 4 * g + q
                    b0 = 32 * q
                    bp = b_ps.tile([P, TG], F32, tag="b")
                    nc.tensor.matmul(
                        bp[:],
                        st_sb[b0:b0 + R1, st * P:(st + 1) * P],
                        ut_sb[b0:b0 + R1, cols],
                        start=True, stop=True,
                        tile_position=(b0, 0),
                    )
                    bps.append(bp)
                for q in range(4):
                    st = 4 * g + q
                    o = out_pool.tile([P, TG], F32, tag="o")
                    if q % 2 == 0:
                        nc.vector.tensor_copy(o[:], bps[q][:])
                    else:
                        nc.scalar.copy(o[:], bps[q][:])
                    nc.sync.dma_start(
                        out[st * P:(st + 1) * P, cols], o[:])

        # scaledT needs groups 0..3; interleave B as soon as its uT exists.
        for tg in range(4):
            stage_a(tg)
        for tg in range(4):
            stage_b(tg)
            stage_a(tg + 4)
        for tg in range(4, 8):
            stage_b(tg)


def _build():
    nc = bacc.Bacc("TRN2", target_bir_lowering=False, debug=False,
                   num_devices=8)
    aps = {}
    decls = [
        ("hst", [H, S], BF16, "ExternalInput"),
        ("wu4", [H, P], BF16, "ExternalInput"),
        ("ws4", [H, P], BF16, "ExternalInput"),
        ("smul4", [P, 1], F32, "ExternalInput"),
        ("sadd4", [P, 1], F32, "ExternalInput"),
        ("out", [SH, S], F32, "ExternalOutput"),
    ]
    for name, shape, dt_, kind in decls:
        aps[name] = nc.dram_tensor(name, shape, dt_, kind=kind).ap()
    with tile.TileContext(nc) as tc:
        _emit(tc, aps)
    nc.compile()
    return nc


_CACHE = {}


def _get_nc(key="bf16"):
    if "nc" not in _CACHE:
        _CACHE["nc"] = _build()
    return _CACHE["nc"]


def _prep_in_maps(hidden_states, wc, bc, we, be, strength, key="bf16"):
    hsf = np.ascontiguousarray(np.asarray(hidden_states, np.float32))
    wc = np.asarray(wc, np.float32)
    bc = np.asarray(bc, np.float32)
    we = np.asarray(we, np.float32)
    be = np.asarray(be, np.float32)
    strength = np.asarray(strength, np.float32)

    wc1 = np.concatenate([wc, np.zeros((1, H), np.float32)], 0)   # [17, H]
    bc1 = np.concatenate([bc, np.ones(1, np.float32)])
    st1 = np.concatenate([strength, np.ones(1, np.float32)])
    we1 = np.concatenate([we, be.sum(0, keepdims=True)], 0)       # [17, H]

    wu4 = np.zeros((H, P), np.float32)
    ws4 = np.zeros((H, P), np.float32)
    smul4 = np.zeros((P, 1), np.float32)
    sadd4 = np.zeros((P, 1), np.float32)
    for i in range(4):
        wu4[:, 32 * i:32 * i + R1] = we1.T
        ws4[:, 32 * i:32 * i + R1] = wc1.T
        smul4[32 * i:32 * i + R1, 0] = st1
        sadd4[32 * i:32 * i + R1, 0] = bc1 * st1

    shared = {
        "wu4": np.ascontiguousarray(wu4.astype(ml_dtypes.bfloat16)),
        "ws4": np.ascontiguousarray(ws4.astype(ml_dtypes.bfloat16)),
        "smul4": smul4,
        "sadd4": sadd4,
    }
    in_maps = []
    for core in range(8):
        b, half = core // 2, core % 2
        hsT = hsf[b].T                                            # [H, S] view
        if half == 1:
            hsT = np.concatenate([hsT[:, SH:], hsT[:, :SH]], 1)
        in_maps.append(
            {"hst": np.ascontiguousarray(hsT.astype(ml_dtypes.bfloat16)),
             **shared})
    return in_maps


def _assemble(results):
    full = np.empty((B, S, S), np.float32)
    for core in range(8):
        b, half = core // 2, core % 2
        o = results[core]["out"]
        if half == 0:
            full[b, :SH, :] = o
        else:
            full[b, SH:, SH:] = o[:, :SH]
            full[b, SH:, :SH] = o[:, SH:]
    return full


def kernel(hidden_states, wc, bc, we, be, strength):
    nc = _get_nc()
    in_maps = _prep_in_maps(hidden_states, wc, bc, we, be, strength)
    res = run_bass_kernel_spmd(nc, in_maps, core_ids=list(range(8)))
    return _assemble(res.results)


def kernel_traced(hidden_states, wc, bc, we, be, strength, key="bf16",
                  **trace_kwargs):
    """Test-harness entry: returns (output, BassKernelResults with trace)."""
    nc = _get_nc(key)
    in_maps = _prep_in_maps(hidden_states, wc, bc, we, be, strength, key)
    res = run_bass_kernel_spmd(nc, in_maps, core_ids=list(range(8)),
                               trace=True, **trace_kwargs)
    return _assemble(res.results), res


# revision 12
# speedup vs baseline: 1.0228x; 1.0228x over previous
"""Trainium2 Bass kernel for CausalBiasingNetwork bias computation.

bias[b,s,t] = sum_r (hs[b,s]@wc_r + bc_r)*strength_r * (hs[b,t]@we_r)
             + hs[b,t] @ be.sum(0)

Folded into a rank-17 form: append rule r=16 with wc=0, bc=1, strength=1,
we=be.sum(0).  Then with
    scaledT[r,s] = (hs[b,s] @ wc'_r + bc'_r) * strength'_r      [17, S]
    uT[r,t]     = hs[b,t] @ we'_r                               [17, S]
    bias[b]     = scaledT.T @ uT                                [S, S]

The K=17 bias matmuls are packed 4-at-a-time into the PE array via
tile_position row-tiling (strips at partitions 0/32/64/96).  To feed the
strips, uT is produced already replicated at all four partition bases
(the u-weights are replicated host-side, so the replication is free in
the A-stage matmul), and scaledT chunk q of each 512-column group is
stored at base 32*q — which is exactly the strip that s-tile uses.

Sharding: 8 cores = 4 batches x 2 sequence halves.  Core (b, h) receives
hs[b]^T (h-major, as the PE contraction needs) rolled so its 2048 output
rows come first; it computes out[s, t_rolled] and the host un-rolls the
columns when assembling the full [4, 4096, 4096] output.
"""

import contextlib

import ml_dtypes
import numpy as np

import concourse.bacc as bacc
import concourse.mybir as mybir
import concourse.tile as tile
from concourse.bass_utils import run_bass_kernel_spmd

B, S, H, R = 4, 4096, 1024, 16
R1 = R + 1          # 17 rules after folding the be-bias term
SH = S // 2         # 2048 output rows per core
P = 128             # partitions
TG = 512            # t-group width (one psum bank of f32)
N_TG = S // TG      # 8 t-groups
N_STILE = SH // P   # 16 s-tiles per core
F32 = mybir.dt.float32
BF16 = mybir.dt.bfloat16


def _emit(tc, aps):
    nc = tc.nc
    hst, wus, ss, out = aps["hst"], aps["wus"], aps["ss"], aps["out"]

    with contextlib.ExitStack() as ctx:
        consts = ctx.enter_context(tc.tile_pool(name="consts", bufs=1))
        hst_pool = ctx.enter_context(tc.tile_pool(name="hst", bufs=24))
        big_pool = ctx.enter_context(tc.tile_pool(name="big", bufs=1))
        out_pool = ctx.enter_context(tc.tile_pool(name="out", bufs=8))
        mm_ps = ctx.enter_context(
            tc.tile_pool(name="mm_ps", bufs=2, space="PSUM"))
        b_ps = ctx.enter_context(
            tc.tile_pool(name="b_ps", bufs=6, space="PSUM"))

        # ---- constants (one DMA for the stacked weights, one for scales) ----
        wus_sb = consts.tile([P, 8 * 2 * P], BF16)  # per chunk: [u 128 | s 128]
        for c in range(8):
            nc.sync.dma_start(wus_sb[:, c * 2 * P:(c + 1) * 2 * P],
                                wus[c * P:(c + 1) * P, :])
        ss_sb = consts.tile([P, 2], F32)            # col 0: smul4, col 1: sadd4
        nc.sync.dma_start(ss_sb[:], ss)
        smul_sb = ss_sb[:, 0:1]
        sadd_sb = ss_sb[:, 1:2]

        def wu_chunk(hc):
            return wus_sb[:, hc * 2 * P:hc * 2 * P + P]

        def ws_chunk(hc):
            return wus_sb[:, hc * 2 * P + P:(hc + 1) * 2 * P]

        ut_sb = big_pool.tile([P, S], BF16)      # uT at bases 0/32/64/96
        st_sb = big_pool.tile([P, SH], BF16)     # scaledT, chunk q at base 32q

        def stage_a(tg):
            """Load hsT tiles for t-group tg; compute uT (+scaledT)."""
            cols = slice(tg * TG, (tg + 1) * TG)
            hsTt = []
            for hc in range(8):
                h = hst_pool.tile([P, TG], BF16, tag="hst")
                nc.sync.dma_start(
                    h[:], hst[hc * P:(hc + 1) * P, cols])
                hsTt.append(h)
            u_ps = mm_ps.tile([P, TG], F32, tag="mm")
            for hc in range(8):
                nc.tensor.matmul(
                    u_ps[:], wu_chunk(hc), hsTt[hc][:],
                    start=(hc == 0), stop=(hc == 7),
                )
            nc.scalar.copy(ut_sb[:, cols], u_ps[:])
            if tg < N_TG // 2:
                s_ps = mm_ps.tile([P, TG], F32, tag="mm")
                for hc in range(8):
                    nc.tensor.matmul(
                        s_ps[:], ws_chunk(hc), hsTt[hc][:],
                        start=(hc == 0), stop=(hc == 7),
                    )
                for q in range(4):
                    b0 = 32 * q
                    st = tg * 4 + q
                    nc.vector.tensor_scalar(
                        st_sb[b0:b0 + R1, st * P:(st + 1) * P],
                        s_ps[b0:b0 + R1, q * P:(q + 1) * P],
                        smul_sb[b0:b0 + R1, :], sadd_sb[b0:b0 + R1, :],
                        mybir.AluOpType.mult, mybir.AluOpType.add,
                    )

        def stage_b(tg, g):
            """One group of 4 packed bias s-tiles for t-group tg + store."""
            cols = slice(tg * TG, (tg + 1) * TG)
            bps = []
            for q in range(4):
                st = 4 * g + q
                b0 = 32 * q
                bp = b_ps.tile([P, TG], F32, tag="b")
                nc.tensor.matmul(
                    bp[:],
                    st_sb[b0:b0 + R1, st * P:(st + 1) * P],
                    ut_sb[b0:b0 + R1, cols],
                    start=True, stop=True,
                    tile_position=(b0, 0),
                )
                bps.append(bp)
            for q in range(4):
                st = 4 * g + q
                o = out_pool.tile([P, TG], F32, tag="o")
                if q % 2 == 0:
                    nc.vector.tensor_copy(o[:], bps[q][:])
                else:
                    nc.scalar.copy(o[:], bps[q][:])
                nc.sync.dma_start(
                    out[st * P:(st + 1) * P, cols], o[:])

        # Emit in readiness order: B(tg, g) needs uT(tg) and scaledT from
        # A(g), so after A(k) every B with max(tg, g) == k is ready.
        for k in range(4):
            stage_a(k)
            for g in range(k):
                stage_b(k, g)
            for tg in range(k + 1):
                stage_b(tg, k)
        for k in range(4, 8):
            stage_a(k)
            for g in range(4):
                stage_b(k, g)


def _build():
    nc = bacc.Bacc("TRN2", target_bir_lowering=False, debug=False,
                   num_devices=8)
    aps = {}
    decls = [
        ("hst", [H, S], BF16, "ExternalInput"),
        ("wus", [H, 2 * P], BF16, "ExternalInput"),
        ("ss", [P, 2], F32, "ExternalInput"),
        ("out", [SH, S], F32, "ExternalOutput"),
    ]
    for name, shape, dt_, kind in decls:
        aps[name] = nc.dram_tensor(name, shape, dt_, kind=kind).ap()
    with tile.TileContext(nc) as tc:
        _emit(tc, aps)
    nc.compile()
    return nc


_CACHE = {}


def _get_nc(key="bf16"):
    if "nc" not in _CACHE:
        _CACHE["nc"] = _build()
    return _CACHE["nc"]


def _prep_in_maps(hidden_states, wc, bc, we, be, strength, key="bf16"):
    hsf = np.ascontiguousarray(np.asarray(hidden_states, np.float32))
    wc = np.asarray(wc, np.float32)
    bc = np.asarray(bc, np.float32)
    we = np.asarray(we, np.float32)
    be = np.asarray(be, np.float32)
    strength = np.asarray(strength, np.float32)

    wc1 = np.concatenate([wc, np.zeros((1, H), np.float32)], 0)   # [17, H]
    bc1 = np.concatenate([bc, np.ones(1, np.float32)])
    st1 = np.concatenate([strength, np.ones(1, np.float32)])
    we1 = np.concatenate([we, be.sum(0, keepdims=True)], 0)       # [17, H]

    wus = np.zeros((H, 2 * P), np.float32)
    ss = np.zeros((P, 2), np.float32)
    for i in range(4):
        wus[:, 32 * i:32 * i + R1] = we1.T
        wus[:, P + 32 * i:P + 32 * i + R1] = wc1.T
        ss[32 * i:32 * i + R1, 0] = st1
        ss[32 * i:32 * i + R1, 1] = bc1 * st1

    shared = {
        "wus": np.ascontiguousarray(wus.astype(ml_dtypes.bfloat16)),
        "ss": ss,
    }
    in_maps = []
    for core in range(8):
        b, half = core // 2, core % 2
        hsT = hsf[b].T                                            # [H, S] view
        if half == 1:
            hsT = np.concatenate([hsT[:, SH:], hsT[:, :SH]], 1)
        in_maps.append(
            {"hst": np.ascontiguousarray(hsT.astype(ml_dtypes.bfloat16)),
             **shared})
    return in_maps


def _assemble(results):
    full = np.empty((B, S, S), np.float32)
    for core in range(8):
        b, half = core // 2, core % 2
        o = results[core]["out"]
        if half == 0:
            full[b, :SH, :] = o
        else:
            full[b, SH:, SH:] = o[:, :SH]
            full[b, SH:, :SH] = o[:, SH:]
    return full


def kernel(hidden_states, wc, bc, we, be, strength):
    nc = _get_nc()
    in_maps = _prep_in_maps(hidden_states, wc, bc, we, be, strength)
    res = run_bass_kernel_spmd(nc, in_maps, core_ids=list(range(8)))
    return _assemble(res.results)


def kernel_traced(hidden_states, wc, bc, we, be, strength, key="bf16",
                  **trace_kwargs):
    """Test-harness entry: returns (output, BassKernelResults with trace)."""
    nc = _get_nc(key)
    in_maps = _prep_in_maps(hidden_states, wc, bc, we, be, strength, key)
    res = run_bass_kernel_spmd(nc, in_maps, core_ids=list(range(8)),
                               trace=True, **trace_kwargs)
    return _assemble(res.results), res
